# revision 71
# baseline (speedup 1.0000x reference)
"""Trainium2 Bass kernel for MultiHeadedAttentionSANM.

Per-core (data-parallel over batch, 8 cores, B=1 each):
  - qkv^T = (x @ Wqkv)^T on PE (float32r): q^T and full v^T (FSMN needs all
    tokens); k^T and a second v^T are computed only for the ~50% of tokens
    with mask=1, gathered host-side into a compact x_c (TK tokens).
  - FSMN: depthwise conv over time in (d, t) layout on DVE (f32), in place
    on v^T with partial-width taps; mask-muls on GPSIMD.
  - attention: scores computed transposed (compact keys on partitions) so the
    exp output feeds the ctx matmul directly as the rhs stream; masked/padded
    keys get a -30000 exp bias. Softmax denominator Z via a ones-weight PE
    pass; normalization is delayed all the way to the out-proj epilogue
    (per-head PSUM + per-partition 1/Z scalars).
"""

import os
import sys

for _p in ("/opt/trn_rl_repo", "/root/.axon_site/_ro/trn_rl_repo"):
    if os.path.isdir(_p) and _p not in sys.path:
        sys.path.append(_p)

from contextlib import ExitStack

import numpy as np

import concourse.bass as bass
import concourse.mybir as mybir
import concourse.tile as tile
from concourse import bacc
from concourse import bass_utils
from concourse.masks import make_identity

T, D, H, DK, KS, PAD = 2048, 512, 4, 128, 11, 5
NCORES = 8
NT = T // 128          # 16 t-blocks of 128
NC = D // 128          # 4 d-chunks of 128
SCALE = float(DK) ** -0.5
MASK_NEG = -30000.0

F32 = mybir.dt.float32
F32R = mybir.dt.float32r
BF16 = mybir.dt.bfloat16
F16 = mybir.dt.float16
AF = mybir.ActivationFunctionType
OP = mybir.AluOpType

QKV_DT = os.environ.get("SANM_QKV_DT", "f32r")   # f32r | f32
ATT_DT = os.environ.get("SANM_ATT_DT", "bf16")   # bf16 | f32
REPS = int(os.environ.get("SANM_REPS", "1"))     # timing: repeat body in one NEFF
LOOP = int(os.environ.get("SANM_LOOP", "0"))     # timing: hw For_i loop trip count
NORM = os.environ.get("SANM_NORM", "psum")       # psum | inplace
FSMN_DT = os.environ.get("SANM_FSMN_DT", "f32")  # f32 | bf16


def _bcast_vec(ap, nrows):
    """Broadcast a flat [N] DRAM AP across partitions -> [nrows, N]."""
    return bass.AP(tensor=ap.tensor, offset=ap.offset, ap=[[0, nrows]] + list(ap.ap))


def _tiles(total, step=512):
    out, p = [], 0
    while p < total:
        n = min(step, total - p)
        rem = total - p - n
        if 0 < rem < 256:  # avoid <256-wide f32r tails (4 cyc/row penalty)
            n = (n + rem) // 2
            n = (n + 127) // 128 * 128
        out.append((p, n))
        p += n
    return out


def build_kernel_body(tc, aps, TK, rep=0):
    nc = tc.nc
    x_d, mask_d, xc_d, cbias_d, wqkv_d, bqkv_d, wout_d, bout_d, fw_d, out_d = aps
    R = f"r{rep}_" if rep else ""
    TKC = TK // 128  # compact key chunks

    stack = ExitStack()
    consts = stack.enter_context(tc.tile_pool(name=R + "consts", bufs=1))
    work = stack.enter_context(tc.tile_pool(name=R + "work", bufs=2))
    ps = stack.enter_context(tc.tile_pool(name=R + "ps", bufs=1, space="PSUM"))

    # p_main holds all long-lived tensors (whole kernel); p_x nests inside it
    # (LIFO) and is released after the qkv matmuls to reclaim x^T space.
    main_cm = tc.tile_pool(name=R + "p_main", bufs=1)
    x_cm = tc.tile_pool(name=R + "p_x", bufs=1)
    p_main = main_cm.__enter__()
    p_x = x_cm.__enter__()

    # ---------------- constants ----------------
    ident = consts.tile([128, 128], F32, name="ident", tag="ident")
    make_identity(nc, ident)
    ident_r = consts.tile([128, 128], F32R, name="ident_r", tag="ident_r")
    nc.vector.tensor_copy(ident_r, ident.bitcast(F32R))
    ident_b = consts.tile([128, 128], BF16, name="ident_b", tag="ident_b")
    nc.vector.tensor_copy(ident_b, ident)
    ident_h = consts.tile([128, 128], F16, name="ident_h", tag="ident_h")
    nc.vector.tensor_copy(ident_h, ident)

    ones_att = consts.tile([128, 1], BF16, name="ones_att", tag="ones_att")
    nc.vector.memset(ones_att, 1.0)
    ones_row = consts.tile([1, 128], BF16, name="ones_row", tag="ones_row")
    nc.vector.memset(ones_row, 1.0)

    # ---------------- x^T and xc^T (XBAR DMA transposes) ---------------------
    # x/xc arrive bf16 (host pre-converts); the DMA engines' 2-byte transpose
    # mode (14ns per 16x128 tile) replaces the load+PE-transpose+copy pipeline.
    # x first (it gates the v projection -> FSMN chain); consts ride SWDGE.
    xT = p_x.tile([128, NC, T], BF16, name="xT", tag="xT")
    xcT = p_x.tile([128, NC, TK], BF16, name="xcT", tag="xcT")
    for c in range(NC):
        (nc.sync if c % 2 == 0 else nc.scalar).dma_start(
            out=xT[:, c, :], in_=x_d[:, c * 128 : (c + 1) * 128], transpose=True
        )

    def xc_transposes():
        for c in range(NC):
            (nc.sync if c % 2 == 0 else nc.scalar).dma_start(
                out=xcT[:, c, :], in_=xc_d[:, c * 128 : (c + 1) * 128], transpose=True
            )

    # biases as per-partition columns (bq gates the first sinks — load first)
    bq = consts.tile([128, 12], F32, name="bq", tag="bq")
    nc.gpsimd.dma_start(out=bq, in_=bqkv_d.rearrange("(c p) -> p c", p=128))

    # remaining consts are emitted just before first use (see load_late_consts)
    mbias = consts.tile([128, TKC], F32, name="mbias", tag="mbias")
    mrow = consts.tile([128, T], BF16, name="mrow", tag="mrow")
    bo = consts.tile([128, NC], F32, name="bo", tag="bo")
    wadj = consts.tile([128, NC, KS], F32, name="wadj", tag="wadj")
    wo = consts.tile([128, NC, D], BF16, name="wo", tag="wo")

    def load_fsmn_consts():
        # mask broadcast (128, T) bf16 (exact for 0/1) + conv weights; needed
        # by the first FSMN pull, after the first v projection
        nc.gpsimd.dma_start(out=mrow, in_=_bcast_vec(mask_d, 128))
        nc.gpsimd.dma_start(
            out=wadj, in_=fw_d.rearrange("(c p) o k -> p c (o k)", p=128)
        )
        nc.vector.tensor_scalar_add(
            wadj[:, :, PAD : PAD + 1], wadj[:, :, PAD : PAD + 1], 1.0
        )
        nc.gpsimd.dma_start(out=bo, in_=bout_d.rearrange("(c p) -> p c", p=128))

    def load_late_consts():
        # compact-key exp bias (0 valid / -30000 padded), columns (128, TKC)
        nc.gpsimd.dma_start(out=mbias, in_=cbias_d.rearrange("(c p) -> p c", p=128))
        # Wout (128, NC, 512) bf16, loaded directly (host pre-converts)
        nc.gpsimd.dma_start(out=wo, in_=wout_d.rearrange("(c p) d -> p c d", p=128))

    # ---------------- qkv^T = (x @ Wqkv)^T ----------------
    # q on full tokens; k only compact; v full (FSMN) and compact (attention)
    qT = [p_main.tile([128, T], BF16, name=f"qT{h}", tag=f"qT{h}") for h in range(H)]
    kTc = [p_main.tile([128, TK], BF16, name=f"kTc{h}", tag=f"kTc{h}") for h in range(H)]
    # fp16 for the FSMN path: same 2-byte DVE speed, 8x finer mantissa (the
    # conv accumulates at the residual's scale, where bf16 rounding is ~1e-2)
    vT = [p_main.tile([128, T], F16, name=f"vT{c}", tag=f"vT{c}") for c in range(NC)]
    vcT = [p_main.tile([128, TK], BF16, name=f"vcT{c}", tag=f"vcT{c}") for c in range(NC)]

    def project(f, srcT, tspans, sink):
        """psum[128, n] = Wqkv[:, f-block].T @ src over d-chunks, then sink.

        Spans are processed in pairs with dc outer, so each stationary load
        serves two matmuls while sinks still only bunch two-deep (ACT keeps
        pace with the PE)."""
        wqf = work.tile([128, NC, 128], BF16, name="wqf", tag="wqf", bufs=3)
        wqf_src = wqkv_d[:, f * 128 : (f + 1) * 128].rearrange(
            "(c p) f -> p c f", p=128
        )
        nc.gpsimd.dma_start(out=wqf, in_=wqf_src)
        for g0 in range(0, len(tspans), 2):
            pair = tspans[g0 : g0 + 2]
            mms = [ps.tile([128, 512], F32, name="mmq", tag="s", bufs=4) for _ in pair]
            for dc in range(NC):
                for mm, (t0, n) in zip(mms, pair):
                    nc.tensor.matmul(
                        mm[:, :n],
                        wqf[:, dc, :],
                        srcT[:, dc, t0 : t0 + n],
                        start=(dc == 0),
                        stop=(dc == NC - 1),
                    )
            for mm, (t0, n) in zip(mms, pair):
                sink(mm, t0, n)

    def act_sink(dst, f):
        def sink(mm, t0, n):
            nc.scalar.activation(
                dst[:, t0 : t0 + n], mm[:, :n], AF.Identity,
                bias=bq[:, f : f + 1], scale=1.0,
            )
        return sink

    # -------- FSMN op stream (d, t layout), fp16, all on DVE -----------------
    # v is projected FIRST so the FSMN conv can run on DVE throughout the
    # PE-heavy q/k/vc projection phase; ops are yielded one at a time and
    # pulled between projection blocks / attention blocks
    vmt = [p_main.tile([128, T], F16, name=f"vmt{c}", tag=f"vmt{c}") for c in range(NC)]
    fac = [p_main.tile([128, T], F16, name=f"fac{c}", tag=f"fac{c}") for c in range(NC)]

    def _fsmn_ops():
        # STT (mult+add) has no fast DVE ucode mode, so each tap is a
        # 4x-mode tensor_scalar mult into a scratch plus a 2x-mode add.
        # Yields one DVE op at a time so the attention loop can interleave
        # them finely and the in-order DVE queue never falls behind.
        for c in range(NC):
            vm, acc = vmt[c], fac[c]
            yield lambda c=c, vm=vm: nc.vector.tensor_tensor(
                vm, vT[c], mrow, op=OP.mult
            )
            yield lambda c=c, vm=vm, acc=acc: nc.vector.tensor_scalar_mul(
                acc, vm, wadj[:, c, PAD : PAD + 1]
            )
            for kk in list(range(0, PAD)) + list(range(PAD + 1, KS)):
                s = kk - PAD
                lo, hi = max(0, -s), T - max(0, s)

                def tapop(c=c, vm=vm, acc=acc, kk=kk, lo=lo, hi=hi, s=s):
                    tap = work.tile([128, T], F16, name="tap", tag="tap", bufs=2)
                    nc.vector.tensor_scalar_mul(
                        tap[:, lo:hi], vm[:, lo + s : hi + s], wadj[:, c, kk : kk + 1]
                    )
                    nc.vector.tensor_tensor(
                        acc[:, lo:hi], acc[:, lo:hi], tap[:, lo:hi], op=OP.add
                    )

                yield tapop
            # out = (conv + vm) * m + bo (center tap carries the +1 residual)
            yield lambda c=c, acc=acc: nc.vector.tensor_tensor(
                vT[c], acc, mrow, op=OP.mult
            )
            yield lambda c=c: nc.vector.tensor_scalar_add(
                vT[c], vT[c], bo[:, c : c + 1]
            )

    fsmn_iter = _fsmn_ops()

    def pull_fsmn(k):
        for _ in range(k):
            op = next(fsmn_iter, None)
            if op is None:
                return
            op()

    for i, f in enumerate(range(8, 12)):  # v full FIRST (unblocks FSMN on DVE)
        project(f, xT, _tiles(T), act_sink(vT[f - 8], f))
        if i == 0:
            load_fsmn_consts()
            xc_transposes()  # off the startup-critical path
        else:
            pull_fsmn(14)  # chunk i-1 (vT[i-1] is fully sinked by now)
    load_late_consts()
    for f in range(4):  # q: full tokens -> qT (bf16), bias via ACT
        project(f, xT, _tiles(T), act_sink(qT[f], f))
        pull_fsmn(4)
    for f in range(4, 8):  # k: compact tokens -> kTc
        project(f, xcT, _tiles(TK), act_sink(kTc[f - 4], f))
    for f in range(8, 12):  # v compact tokens (attention)
        project(f, xcT, _tiles(TK), act_sink(vcT[f - 8], f))
    x_cm.__exit__(None, None, None)  # frees xT, xcT
    pull_fsmn(99)  # any remainder: epilogue blocks interleave with attention

    # ------------- compact v natural (XBAR DMA transposes of vcT) ------------
    vh = [
        p_main.tile([128, TKC, 128], BF16, name=f"vh{h}", tag=f"vh{h}")
        for h in range(H)
    ]
    for h in range(H):
        for jc in range(TKC):
            (nc.sync if (h + jc) % 2 == 0 else nc.scalar).dma_start(
                out=vh[h][:, jc, :],
                in_=vcT[h][:, jc * 128 : (jc + 1) * 128],
                transpose=True,
            )

    # ---------------- attention + interleaved epilogue ----------------
    # per (query-block of 512, head): scores transposed (compact keys on
    # partitions); exp with -30000 pad bias; ctx accumulates over key chunks
    # in one PSUM bank; Z = sum_k exp via a DVE/Pool add tree + one
    # ones-matmul; 1/Z broadcast across partitions via a DRAM bounce;
    # normalize on DVE. qb is the OUTER loop so each query block's out
    # projection runs as soon as its 4 heads are done (no serial tail).
    ctxT = [
        p_main.tile([128, T], BF16, name=f"ctxT{h}", tag=f"ctxT{h}")
        for h in range(H)
    ]

    def att_pair(h, qp):
        """Two 512-query blocks share every PE stationary load (kTc, vh,
        ones): halves the attention ldweights count."""
        ia, ib = qp * 1024, qp * 1024 + 512
        ctx_a = ps.tile([128, 512], F32, name="ctx_a", tag="actx", bufs=3)
        ctx_b = ps.tile([128, 512], F32, name="ctx_b", tag="actx", bufs=3)
        esum_a = work.tile([128, 512], BF16, name="esum_a", tag="esum", bufs=4)
        esum_b = work.tile([128, 512], BF16, name="esum_b", tag="esum", bufs=4)
        last_e = []
        for jc in range(TKC):
            kT_j = kTc[h][:, jc * 128 : (jc + 1) * 128]
            s_a = ps.tile([128, 512], F32, name="s_a", tag="s", bufs=4)
            s_b = ps.tile([128, 512], F32, name="s_b", tag="s", bufs=4)
            nc.tensor.matmul(
                s_a, kT_j, qT[h][:, ia : ia + 512],
                start=True, stop=True, skip_group_check=True,
            )
            nc.tensor.matmul(
                s_b, kT_j, qT[h][:, ib : ib + 512],
                start=True, stop=True, skip_group_check=True,
            )
            e_a = work.tile([128, 512], BF16, name="e_a", tag="eT", bufs=4)
            e_b = work.tile([128, 512], BF16, name="e_b", tag="eT", bufs=4)
            nc.scalar.activation(
                e_a, s_a, AF.Exp, bias=mbias[:, jc : jc + 1], scale=SCALE
            )
            nc.scalar.activation(
                e_b, s_b, AF.Exp, bias=mbias[:, jc : jc + 1], scale=SCALE
            )
            vh_j = vh[h][:, jc, :]
            nc.tensor.matmul(
                ctx_a, vh_j, e_a,
                start=(jc == 0), stop=(jc == TKC - 1), skip_group_check=True,
            )
            nc.tensor.matmul(
                ctx_b, vh_j, e_b,
                start=(jc == 0), stop=(jc == TKC - 1), skip_group_check=True,
            )
            # Z = sum_k exp: DVE accumulates the first 7 chunks (bf16 2x mode;
            # attention is ~1% of |out| so bf16 partials are plenty), the last
            # two ride PE ones-matmuls below to balance engine load
            if jc == 0:
                nc.vector.tensor_copy(esum_a, e_a)
                nc.vector.tensor_copy(esum_b, e_b)
            elif jc < TKC - 2:
                nc.vector.tensor_tensor(esum_a, esum_a, e_a, op=OP.add)
                nc.vector.tensor_tensor(esum_b, esum_b, e_b, op=OP.add)
            else:
                last_e.append((e_a, e_b))
        for half, (i0, esum_d) in enumerate(((ia, esum_a), (ib, esum_b))):
            z_ps = ps.tile([1, 512], F32, name="z_ps", tag="z", bufs=1)
            for i, e_pair in enumerate(last_e):
                nc.tensor.matmul(
                    z_ps, ones_att, e_pair[half],
                    start=(i == 0), stop=False, skip_group_check=True,
                )
            nc.tensor.matmul(
                z_ps, ones_att, esum_d, start=False, stop=True,
                skip_group_check=True,
            )
            rz = work.tile([1, 512], BF16, name="rz", tag="rz", bufs=2)
            with nc.allow_low_precision(reason="1/Z applied to bf16 weights"):
                nc.vector.reciprocal(rz, z_ps)
            # broadcast 1/Z across partitions as a PE outer product; shares
            # the z bank (the z->recip->broadcast chain is strictly serial)
            zb_ps = ps.tile([128, 512], F32, name="zb_ps", tag="z", bufs=1)
            nc.tensor.matmul(
                zb_ps, ones_row, rz, start=True, stop=True, skip_group_check=True
            )
            zb_sb = work.tile([128, 512], BF16, name="zb_sb", tag="zb_sb", bufs=2)
            nc.scalar.copy(zb_sb, zb_ps)
            ctx = ctx_a if half == 0 else ctx_b
            nc.vector.tensor_tensor(
                ctxT[h][:, i0 : i0 + 512], ctx, zb_sb, op=OP.mult
            )

    def out_block(tb):
        # transpose this t-block of fsmn into natural layout via XBAR DMA
        # (issued first: independent of the matmuls, latency hides behind them)
        f_sb = work.tile([128, D], F16, name="f_sb", tag="f_sb", bufs=4)
        for c in range(NC):
            (nc.sync if (tb + c) % 2 == 0 else nc.scalar).dma_start(
                out=f_sb[:, c * 128 : (c + 1) * 128],
                in_=vT[c][:, tb * 128 : (tb + 1) * 128],
                transpose=True,
            )
        op_ps = ps.tile([128, 512], F32, name="op_ps", tag="actx", bufs=3)
        for h in range(H):
            nc.tensor.matmul(
                op_ps,
                ctxT[h][:, tb * 128 : (tb + 1) * 128],
                wo[:, h, :],
                start=(h == 0),
                stop=(h == H - 1),
            )
        o_sb = work.tile([128, D], F32, name="o_sb", tag="o_sb", bufs=2)
        nc.vector.tensor_tensor(o_sb, op_ps, f_sb, op=OP.add)
        nc.sync.dma_start(out=out_d[tb * 128 : (tb + 1) * 128, :], in_=o_sb)

    for qp in range(2):  # query-block pairs of 1024
        for h in range(H):
            att_pair(h, qp)
        if os.environ.get("SANM_EPI", "end") == "interleave":
            for tb8 in range(8):  # this pair's 128-token out blocks
                out_block(qp * 8 + tb8)
    if os.environ.get("SANM_EPI", "end") != "interleave":
        for tb in range(NT):
            out_block(tb)

    if os.environ.get("SANM_DEBUG", "0") == "1":
        dbg_q = nc.dram_tensor("dbg_q", (H, 128, T), BF16, kind="ExternalOutput").ap()
        dbg_v = nc.dram_tensor("dbg_v", (NC, 128, T), F16, kind="ExternalOutput").ap()
        dbg_c = nc.dram_tensor("dbg_c", (H, 128, T), BF16, kind="ExternalOutput").ap()
        dbg_k = nc.dram_tensor("dbg_k", (H, 128, TK), BF16, kind="ExternalOutput").ap()
        for hh in range(H):
            nc.sync.dma_start(out=dbg_q[hh], in_=qT[hh])
            nc.sync.dma_start(out=dbg_c[hh], in_=ctxT[hh])
            nc.sync.dma_start(out=dbg_k[hh], in_=kTc[hh])
            nc.sync.dma_start(out=dbg_v[hh], in_=vT[hh])

    main_cm.__exit__(None, None, None)
    stack.close()


_CACHE = {}
_FN_CACHE = {}


def make_sharded_fn(nc, n_cores=NCORES):
    """Build a reusable jitted executable for `nc` (done once per build).

    run_bass_kernel_spmd creates a fresh jax.jit per call, so every
    invocation re-traces, re-lowers and re-loads the NEFF; caching the
    jitted callable makes repeat kernel() calls cost only transfer+exec.
    """
    import jax
    from jax.experimental.shard_map import shard_map
    from jax.sharding import Mesh, PartitionSpec

    from concourse import bass2jax
    from concourse.bass2jax import _bass_exec_p, install_neuronx_cc_hook

    install_neuronx_cc_hook()
    partition_name = nc.partition_id_tensor.name if nc.partition_id_tensor else None
    in_names, out_names, out_avals, zero_outs = [], [], [], []
    for alloc in nc.m.functions[0].allocations:
        if not isinstance(alloc, mybir.MemoryLocationSet):
            continue
        name = alloc.memorylocations[0].name
        if alloc.kind == "ExternalInput":
            if name != partition_name:
                in_names.append(name)
        elif alloc.kind == "ExternalOutput":
            out_names.append(name)
            shape = tuple(alloc.tensor_shape)
            dtype = mybir.dt.np(alloc.dtype)
            out_avals.append(jax.core.ShapedArray(shape, dtype))
            zero_outs.append(np.zeros(shape, dtype))
    n_params = len(in_names)
    all_in_names = list(in_names) + list(out_names)
    if partition_name is not None:
        all_in_names.append(partition_name)

    def _body(*args):
        operands = list(args)
        if partition_name is not None:
            operands.append(bass2jax.partition_id_tensor())
        outs = _bass_exec_p.bind(
            *operands,
            out_avals=tuple(out_avals),
            in_names=tuple(all_in_names),
            out_names=tuple(out_names),
            lowering_input_output_aliases=(),
            sim_require_finite=True,
            sim_require_nnan=True,
            nc=nc,
        )
        return tuple(outs)

    devices = jax.devices()[:n_cores]
    mesh = Mesh(np.asarray(devices), ("core",))
    n_outs = len(out_avals)
    in_specs = (PartitionSpec("core"),) * (n_params + n_outs)
    out_specs = (PartitionSpec("core"),) * n_outs
    fn = jax.jit(
        shard_map(
            _body, mesh=mesh, in_specs=in_specs, out_specs=out_specs, check_rep=False
        ),
        keep_unused=True,
    )
    return fn, in_names, out_names, zero_outs


def run_cached(nc, in_maps, key):
    """Execute via a cached jitted executable (falls back to the slow path)."""
    import jax

    if key not in _FN_CACHE:
        _FN_CACHE[key] = make_sharded_fn(nc)
    fn, in_names, out_names, zero_outs = _FN_CACHE[key]
    n = len(in_maps)
    concat_in = [
        np.concatenate([np.asarray(in_maps[c][name]) for c in range(n)], axis=0)
        for name in in_names
    ]
    concat_zeros = [
        np.zeros((n * z.shape[0], *z.shape[1:]), z.dtype) for z in zero_outs
    ]
    out_arrs = fn(*concat_in, *concat_zeros)
    outs = [np.asarray(a) for a in out_arrs]
    return [
        {
            name: outs[i].reshape(n, outs[i].shape[0] // n, *outs[i].shape[1:])[c]
            for i, name in enumerate(out_names)
        }
        for c in range(n)
    ]


def _build(TK):
    key = (QKV_DT, ATT_DT, REPS, TK, NORM, FSMN_DT, LOOP)
    if key in _CACHE:
        return _CACHE[key]
    nc = bacc.Bacc(
        "TRN2",
        target_bir_lowering=False,
        debug=False,
        enable_asserts=False,
        num_devices=NCORES,
    )
    aps = (
        nc.dram_tensor("x", (T, D), BF16, kind="ExternalInput").ap(),
        nc.dram_tensor("mask", (T,), F32, kind="ExternalInput").ap(),
        nc.dram_tensor("xc", (TK, D), BF16, kind="ExternalInput").ap(),
        nc.dram_tensor("cbias", (TK,), F32, kind="ExternalInput").ap(),
        nc.dram_tensor("Wqkv", (D, 3 * D), BF16, kind="ExternalInput").ap(),
        nc.dram_tensor("bqkv", (3 * D,), F32, kind="ExternalInput").ap(),
        nc.dram_tensor("Wout", (D, D), BF16, kind="ExternalInput").ap(),
        nc.dram_tensor("bout", (D,), F32, kind="ExternalInput").ap(),
        nc.dram_tensor("fsmn_w", (D, 1, KS), F32, kind="ExternalInput").ap(),
        nc.dram_tensor("out", (T, D), F32, kind="ExternalOutput").ap(),
    )
    with tile.TileContext(nc) as tc:
        if LOOP > 0:
            # hw loop: NEFF size is constant in trip count, so a large trip
            # count isolates per-rep device time from dispatch overhead
            with tc.For_i(0, LOOP, 1):
                build_kernel_body(tc, aps, TK, 0)
        else:
            for rep in range(REPS):
                build_kernel_body(tc, aps, TK, rep)
    nc.compile()
    _CACHE[key] = nc
    return nc


def _bf16(a):
    import ml_dtypes

    return np.ascontiguousarray(a.astype(ml_dtypes.bfloat16))


def _compact(x_b, mask_b, TK):
    """Host-side gather of unmasked token rows, padded to TK (bf16 in/out)."""
    idx = np.nonzero(mask_b != 0)[0]
    n = len(idx)
    xc = np.zeros((TK, x_b.shape[1]), x_b.dtype)
    xc[:n] = x_b[idx[:TK]]
    cb = np.full((TK,), MASK_NEG, np.float32)
    cb[:n] = 0.0
    return xc, cb


def kernel(x, mask, Wqkv, bqkv, Wout, bout, fsmn_w):
    x = _bf16(np.asarray(x))
    mask = np.ascontiguousarray(np.asarray(mask, dtype=np.float32))
    Wqkv = _bf16(np.asarray(Wqkv))
    bqkv = np.ascontiguousarray(np.asarray(bqkv, dtype=np.float32))
    Wout = _bf16(np.asarray(Wout))
    bout = np.ascontiguousarray(np.asarray(bout, dtype=np.float32))
    fsmn_w = np.ascontiguousarray(np.asarray(fsmn_w, dtype=np.float32))

    counts = [int((mask[b, 0] != 0).sum()) for b in range(NCORES)]
    TK = min(T, max(256, int(-(-max(counts) // 128) * 128)))

    nc = _build(TK)
    in_maps = []
    for b in range(NCORES):
        xc, cb = _compact(x[b], mask[b, 0], TK)
        in_maps.append(
            {
                "x": x[b],
                "mask": np.ascontiguousarray(mask[b, 0]),
                "xc": xc,
                "cbias": cb,
                "Wqkv": Wqkv,
                "bqkv": bqkv,
                "Wout": Wout,
                "bout": bout,
                "fsmn_w": fsmn_w,
            }
        )
    try:
        results = run_cached(nc, in_maps, key=(id(nc), TK))
    except Exception:
        res = bass_utils.run_bass_kernel_spmd(
            nc, in_maps, core_ids=list(range(NCORES)), trace=False
        )
        results = res.results
    out = np.stack([results[b]["out"] for b in range(NCORES)], axis=0)
    return out


if __name__ == "__main__":
    rng = np.random.default_rng(0)
    ins = {
        "x": rng.standard_normal((NCORES, T, D), dtype=np.float32),
        "mask": rng.integers(0, 2, (NCORES, 1, T)).astype(np.float32),
        "Wqkv": (rng.standard_normal((D, 3 * D)) * 0.02).astype(np.float32),
        "bqkv": np.zeros((3 * D,), np.float32),
        "Wout": (rng.standard_normal((D, D)) * 0.02).astype(np.float32),
        "bout": np.zeros((D,), np.float32),
        "fsmn_w": (rng.standard_normal((D, 1, KS)) * 0.1).astype(np.float32),
    }
    out = kernel(**ins)
    print(out.shape, out.dtype, float(np.abs(out).max()))



# revision 77
# speedup vs baseline: 1.1822x; 1.1822x over previous
"""Trainium2 Bass kernel for MultiHeadedAttentionSANM.

Per-core (data-parallel over batch, 8 cores, B=1 each):
  - qkv^T = (x @ Wqkv)^T on PE (float32r): q^T and full v^T (FSMN needs all
    tokens); k^T and a second v^T are computed only for the ~50% of tokens
    with mask=1, gathered host-side into a compact x_c (TK tokens).
  - FSMN: depthwise conv over time in (d, t) layout on DVE (f32), in place
    on v^T with partial-width taps; mask-muls on GPSIMD.
  - attention: scores computed transposed (compact keys on partitions) so the
    exp output feeds the ctx matmul directly as the rhs stream; masked/padded
    keys get a -30000 exp bias. Softmax denominator Z via a ones-weight PE
    pass; normalization is delayed all the way to the out-proj epilogue
    (per-head PSUM + per-partition 1/Z scalars).
"""

import os
import sys

for _p in ("/opt/trn_rl_repo", "/root/.axon_site/_ro/trn_rl_repo"):
    if os.path.isdir(_p) and _p not in sys.path:
        sys.path.append(_p)

from contextlib import ExitStack

import numpy as np

import concourse.bass as bass
import concourse.mybir as mybir
import concourse.tile as tile
from concourse import bacc
from concourse import bass_utils
from concourse.masks import make_identity

T, D, H, DK, KS, PAD = 2048, 512, 4, 128, 11, 5
NCORES = 8
NT = T // 128          # 16 t-blocks of 128
NC = D // 128          # 4 d-chunks of 128
SCALE = float(DK) ** -0.5
MASK_NEG = -30000.0

F32 = mybir.dt.float32
F32R = mybir.dt.float32r
BF16 = mybir.dt.bfloat16
F16 = mybir.dt.float16
AF = mybir.ActivationFunctionType
OP = mybir.AluOpType

REPS = int(os.environ.get("SANM_REPS", "1"))     # timing: repeat body in one NEFF
LOOP = int(os.environ.get("SANM_LOOP", "0"))     # timing: hw For_i loop trip count


def _bcast_vec(ap, nrows):
    """Broadcast a flat [N] DRAM AP across partitions -> [nrows, N]."""
    return bass.AP(tensor=ap.tensor, offset=ap.offset, ap=[[0, nrows]] + list(ap.ap))


def _tiles(total, step=512):
    out, p = [], 0
    while p < total:
        n = min(step, total - p)
        rem = total - p - n
        if 0 < rem < 256:  # avoid <256-wide f32r tails (4 cyc/row penalty)
            n = (n + rem) // 2
            n = (n + 127) // 128 * 128
        out.append((p, n))
        p += n
    return out


def build_kernel_body(tc, aps, TK, rep=0):
    nc = tc.nc
    x_d, mask_d, xc_d, cbias_d, wqkv_d, bqkv_d, wout_d, bout_d, fw_d, out_d = aps
    R = f"r{rep}_" if rep else ""
    TKC = TK // 128  # compact key chunks

    stack = ExitStack()
    consts = stack.enter_context(tc.tile_pool(name=R + "consts", bufs=1))
    work = stack.enter_context(tc.tile_pool(name=R + "work", bufs=2))
    ps = stack.enter_context(tc.tile_pool(name=R + "ps", bufs=1, space="PSUM"))

    # p_main holds all long-lived tensors (whole kernel); p_x nests inside it
    # (LIFO) and is released after the qkv matmuls to reclaim x^T space.
    main_cm = tc.tile_pool(name=R + "p_main", bufs=1)
    x_cm = tc.tile_pool(name=R + "p_x", bufs=1)
    p_main = main_cm.__enter__()
    p_x = x_cm.__enter__()

    # ---------------- constants ----------------
    ident = consts.tile([128, 128], F32, name="ident", tag="ident")
    make_identity(nc, ident)
    ident_b = consts.tile([128, 128], BF16, name="ident_b", tag="ident_b")
    nc.vector.tensor_copy(ident_b, ident)
    ident_h = consts.tile([128, 128], F16, name="ident_h", tag="ident_h")
    nc.vector.tensor_copy(ident_h, ident)

    ones_att = consts.tile([128, 1], BF16, name="ones_att", tag="ones_att")
    nc.vector.memset(ones_att, 1.0)
    ones_row = consts.tile([1, 128], BF16, name="ones_row", tag="ones_row")
    nc.vector.memset(ones_row, 1.0)

    # ---------------- x^T and xc^T (XBAR DMA transposes) ---------------------
    # x/xc arrive bf16 (host pre-converts); the DMA engines' 2-byte transpose
    # mode (14ns per 16x128 tile) replaces the load+PE-transpose+copy pipeline.
    # x first (it gates the v projection -> FSMN chain); consts ride SWDGE.
    xT = p_x.tile([128, NC, T], BF16, name="xT", tag="xT")
    xcT = p_x.tile([128, NC, TK], BF16, name="xcT", tag="xcT")
    for c in range(NC):
        (nc.sync if c % 2 == 0 else nc.scalar).dma_start(
            out=xT[:, c, :], in_=x_d[:, c * 128 : (c + 1) * 128], transpose=True
        )

    def xc_transposes():
        for c in range(NC):
            (nc.sync if c % 2 == 0 else nc.scalar).dma_start(
                out=xcT[:, c, :], in_=xc_d[:, c * 128 : (c + 1) * 128], transpose=True
            )

    # biases as per-partition columns (bq gates the first sinks — load first;
    # HWDGE, not SWDGE: Q7 descriptor generation costs ~15us at startup)
    bq = consts.tile([128, 12], F32, name="bq", tag="bq")
    nc.sync.dma_start(out=bq, in_=bqkv_d.rearrange("(c p) -> p c", p=128))

    # remaining consts are emitted just before first use (see load_late_consts)
    mbias = consts.tile([128, TKC], F32, name="mbias", tag="mbias")
    mrow = consts.tile([128, T], BF16, name="mrow", tag="mrow")
    bo = consts.tile([128, NC], F32, name="bo", tag="bo")
    wadj = consts.tile([128, NC, KS], F32, name="wadj", tag="wadj")
    wo = consts.tile([128, NC, D], BF16, name="wo", tag="wo")

    def load_fsmn_consts():
        # mask broadcast (128, T) bf16 (exact for 0/1) + conv weights; needed
        # by the first FSMN pull, after the first v projection
        nc.gpsimd.dma_start(out=mrow, in_=_bcast_vec(mask_d, 128))
        nc.gpsimd.dma_start(
            out=wadj, in_=fw_d.rearrange("(c p) o k -> p c (o k)", p=128)
        )
        nc.vector.tensor_scalar_add(
            wadj[:, :, PAD : PAD + 1], wadj[:, :, PAD : PAD + 1], 1.0
        )
        nc.gpsimd.dma_start(out=bo, in_=bout_d.rearrange("(c p) -> p c", p=128))

    def load_late_consts():
        # compact-key exp bias (0 valid / -30000 padded), columns (128, TKC)
        nc.gpsimd.dma_start(out=mbias, in_=cbias_d.rearrange("(c p) -> p c", p=128))
        # Wout (128, NC, 512) bf16, loaded directly (host pre-converts)
        nc.gpsimd.dma_start(out=wo, in_=wout_d.rearrange("(c p) d -> p c d", p=128))

    # ---------------- qkv^T = (x @ Wqkv)^T ----------------
    # q on full tokens; k only compact; v full (FSMN) and compact (attention)
    qT = [p_main.tile([128, T], BF16, name=f"qT{h}", tag=f"qT{h}") for h in range(H)]
    kTc = [p_main.tile([128, TK], BF16, name=f"kTc{h}", tag=f"kTc{h}") for h in range(H)]
    # fp16 for the FSMN path: same 2-byte DVE speed, 8x finer mantissa (the
    # conv accumulates at the residual's scale, where bf16 rounding is ~1e-2)
    vT = [p_main.tile([128, T], F16, name=f"vT{c}", tag=f"vT{c}") for c in range(NC)]
    vcT = [p_main.tile([128, TK], BF16, name=f"vcT{c}", tag=f"vcT{c}") for c in range(NC)]

    def project(f, srcT, tspans, sink):
        """psum[128, n] = Wqkv[:, f-block].T @ src over d-chunks, then sink.

        Spans are processed in pairs with dc outer, so each stationary load
        serves two matmuls while sinks still only bunch two-deep (ACT keeps
        pace with the PE)."""
        wqf = work.tile([128, NC, 128], BF16, name="wqf", tag="wqf", bufs=3)
        wqf_src = wqkv_d[:, f * 128 : (f + 1) * 128].rearrange(
            "(c p) f -> p c f", p=128
        )
        (nc.scalar if f % 2 else nc.sync).dma_start(out=wqf, in_=wqf_src)
        for g0 in range(0, len(tspans), 2):
            pair = tspans[g0 : g0 + 2]
            mms = [ps.tile([128, 512], F32, name="mmq", tag="s", bufs=4) for _ in pair]
            for dc in range(NC):
                for mm, (t0, n) in zip(mms, pair):
                    nc.tensor.matmul(
                        mm[:, :n],
                        wqf[:, dc, :],
                        srcT[:, dc, t0 : t0 + n],
                        start=(dc == 0),
                        stop=(dc == NC - 1),
                    )
            for mm, (t0, n) in zip(mms, pair):
                sink(mm, t0, n)

    def act_sink(dst, f):
        def sink(mm, t0, n):
            nc.scalar.activation(
                dst[:, t0 : t0 + n], mm[:, :n], AF.Identity,
                bias=bq[:, f : f + 1], scale=1.0,
            )
        return sink

    # -------- FSMN op stream (d, t layout), fp16, all on DVE -----------------
    # v is projected FIRST so the FSMN conv can run on DVE throughout the
    # PE-heavy q/k/vc projection phase; ops are yielded one at a time and
    # pulled between projection blocks / attention blocks
    vmt = [p_main.tile([128, T], F16, name=f"vmt{c}", tag=f"vmt{c}") for c in range(NC)]
    fac = [p_main.tile([128, T], F16, name=f"fac{c}", tag=f"fac{c}") for c in range(NC)]

    def _fsmn_ops():
        # STT (mult+add) has no fast DVE ucode mode, so each tap is a
        # 4x-mode tensor_scalar mult into a scratch plus a 2x-mode add.
        # Yields one DVE op at a time so the attention loop can interleave
        # them finely and the in-order DVE queue never falls behind.
        for c in range(NC):
            vm, acc = vmt[c], fac[c]
            yield lambda c=c, vm=vm: nc.vector.tensor_tensor(
                vm, vT[c], mrow, op=OP.mult
            )
            yield lambda c=c, vm=vm, acc=acc: nc.vector.tensor_scalar_mul(
                acc, vm, wadj[:, c, PAD : PAD + 1]
            )
            for kk in list(range(0, PAD)) + list(range(PAD + 1, KS)):
                s = kk - PAD
                lo, hi = max(0, -s), T - max(0, s)

                def tapop(c=c, vm=vm, acc=acc, kk=kk, lo=lo, hi=hi, s=s):
                    tap = work.tile([128, T], F16, name="tap", tag="tap", bufs=2)
                    nc.vector.tensor_scalar_mul(
                        tap[:, lo:hi], vm[:, lo + s : hi + s], wadj[:, c, kk : kk + 1]
                    )
                    nc.vector.tensor_tensor(
                        acc[:, lo:hi], acc[:, lo:hi], tap[:, lo:hi], op=OP.add
                    )

                yield tapop
            # out = (conv + vm) * m + bo (center tap carries the +1 residual)
            yield lambda c=c, acc=acc: nc.vector.tensor_tensor(
                vT[c], acc, mrow, op=OP.mult
            )
            yield lambda c=c: nc.vector.tensor_scalar_add(
                vT[c], vT[c], bo[:, c : c + 1]
            )

    fsmn_iter = _fsmn_ops()

    def pull_fsmn(k):
        for _ in range(k):
            op = next(fsmn_iter, None)
            if op is None:
                return
            op()

    for i, f in enumerate(range(8, 12)):  # v full FIRST (unblocks FSMN on DVE)
        project(f, xT, _tiles(T), act_sink(vT[f - 8], f))
        if i == 0:
            load_fsmn_consts()
            xc_transposes()  # off the startup-critical path
        else:
            pull_fsmn(14)  # chunk i-1 (vT[i-1] is fully sinked by now)
    load_late_consts()
    for f in range(4):  # q: full tokens -> qT (bf16), bias via ACT
        project(f, xT, _tiles(T), act_sink(qT[f], f))
        pull_fsmn(4)
    for f in range(4, 8):  # k: compact tokens -> kTc
        project(f, xcT, _tiles(TK), act_sink(kTc[f - 4], f))
    for f in range(8, 12):  # v compact tokens (attention)
        project(f, xcT, _tiles(TK), act_sink(vcT[f - 8], f))
    x_cm.__exit__(None, None, None)  # frees xT, xcT
    pull_fsmn(99)  # any remainder: epilogue blocks interleave with attention

    # ------------- compact v natural (PE transposes of vcT, batched) ---------
    vh = [
        p_main.tile([128, TKC, 128], BF16, name=f"vh{h}", tag=f"vh{h}")
        for h in range(H)
    ]
    for h in range(H):
        for j0 in range(0, TKC, 4):
            jn = min(4, TKC - j0)
            tp = ps.tile([128, 512], BF16, name="tpv", tag="s", bufs=4)
            for j in range(jn):
                nc.tensor.transpose(
                    tp[:, j * 128 : (j + 1) * 128],
                    vcT[h][:, (j0 + j) * 128 : (j0 + j + 1) * 128],
                    ident_b,
                )
            nc.scalar.copy(vh[h][:, j0 : j0 + jn, :], tp[:, : jn * 128])

    # ---------------- attention + interleaved epilogue ----------------
    # per (query-block of 512, head): scores transposed (compact keys on
    # partitions); exp with -30000 pad bias; ctx accumulates over key chunks
    # in one PSUM bank; Z = sum_k exp via a DVE/Pool add tree + one
    # ones-matmul; 1/Z broadcast across partitions via a DRAM bounce;
    # normalize on DVE. qb is the OUTER loop so each query block's out
    # projection runs as soon as its 4 heads are done (no serial tail).
    ctxT = [
        p_main.tile([128, T], BF16, name=f"ctxT{h}", tag=f"ctxT{h}")
        for h in range(H)
    ]

    def att_pair(h, qp):
        """Two 512-query blocks share every PE stationary load (kTc, vh,
        ones): halves the attention ldweights count."""
        ia, ib = qp * 1024, qp * 1024 + 512
        ctx_a = ps.tile([128, 512], F32, name="ctx_a", tag="actx", bufs=3)
        ctx_b = ps.tile([128, 512], F32, name="ctx_b", tag="actx", bufs=3)
        esum_a = work.tile([128, 512], BF16, name="esum_a", tag="esum", bufs=4)
        esum_b = work.tile([128, 512], BF16, name="esum_b", tag="esum", bufs=4)
        last_e = []
        for jc in range(TKC):
            kT_j = kTc[h][:, jc * 128 : (jc + 1) * 128]
            s_a = ps.tile([128, 512], F32, name="s_a", tag="s", bufs=4)
            s_b = ps.tile([128, 512], F32, name="s_b", tag="s", bufs=4)
            nc.tensor.matmul(
                s_a, kT_j, qT[h][:, ia : ia + 512],
                start=True, stop=True, skip_group_check=True,
            )
            nc.tensor.matmul(
                s_b, kT_j, qT[h][:, ib : ib + 512],
                start=True, stop=True, skip_group_check=True,
            )
            e_a = work.tile([128, 512], BF16, name="e_a", tag="eT", bufs=4)
            e_b = work.tile([128, 512], BF16, name="e_b", tag="eT", bufs=4)
            nc.scalar.activation(
                e_a, s_a, AF.Exp, bias=mbias[:, jc : jc + 1], scale=SCALE
            )
            nc.scalar.activation(
                e_b, s_b, AF.Exp, bias=mbias[:, jc : jc + 1], scale=SCALE
            )
            vh_j = vh[h][:, jc, :]
            nc.tensor.matmul(
                ctx_a, vh_j, e_a,
                start=(jc == 0), stop=(jc == TKC - 1), skip_group_check=True,
            )
            nc.tensor.matmul(
                ctx_b, vh_j, e_b,
                start=(jc == 0), stop=(jc == TKC - 1), skip_group_check=True,
            )
            # Z = sum_k exp: DVE accumulates the first 7 chunks (bf16 2x mode;
            # attention is ~1% of |out| so bf16 partials are plenty), the last
            # two ride PE ones-matmuls below to balance engine load
            if jc == 0:
                nc.vector.tensor_copy(esum_a, e_a)
                nc.vector.tensor_copy(esum_b, e_b)
            elif jc < TKC - 2:
                nc.vector.tensor_tensor(esum_a, esum_a, e_a, op=OP.add)
                nc.vector.tensor_tensor(esum_b, esum_b, e_b, op=OP.add)
            else:
                last_e.append((e_a, e_b))
        for half, (i0, esum_d) in enumerate(((ia, esum_a), (ib, esum_b))):
            z_ps = ps.tile([1, 512], F32, name="z_ps", tag="z", bufs=1)
            for i, e_pair in enumerate(last_e):
                nc.tensor.matmul(
                    z_ps, ones_att, e_pair[half],
                    start=(i == 0), stop=False, skip_group_check=True,
                )
            nc.tensor.matmul(
                z_ps, ones_att, esum_d, start=False, stop=True,
                skip_group_check=True,
            )
            rz = work.tile([1, 512], BF16, name="rz", tag="rz", bufs=2)
            with nc.allow_low_precision(reason="1/Z applied to bf16 weights"):
                nc.vector.reciprocal(rz, z_ps)
            # broadcast 1/Z across partitions as a PE outer product; shares
            # the z bank (the z->recip->broadcast chain is strictly serial)
            zb_ps = ps.tile([128, 512], F32, name="zb_ps", tag="z", bufs=1)
            nc.tensor.matmul(
                zb_ps, ones_row, rz, start=True, stop=True, skip_group_check=True
            )
            zb_sb = work.tile([128, 512], BF16, name="zb_sb", tag="zb_sb", bufs=2)
            nc.scalar.copy(zb_sb, zb_ps)
            ctx = ctx_a if half == 0 else ctx_b
            nc.vector.tensor_tensor(
                ctxT[h][:, i0 : i0 + 512], ctx, zb_sb, op=OP.mult
            )

    def out_block(tb):
        op_ps = ps.tile([128, 512], F32, name="op_ps", tag="actx", bufs=3)
        for h in range(H):
            nc.tensor.matmul(
                op_ps,
                ctxT[h][:, tb * 128 : (tb + 1) * 128],
                wo[:, h, :],
                start=(h == 0),
                stop=(h == H - 1),
            )
        # transpose this t-block of fsmn into natural layout (fp16, 1 cyc/row)
        ftp = ps.tile([128, 512], F16, name="ftp", tag="z", bufs=1)
        for c in range(NC):
            nc.tensor.transpose(
                ftp[:, c * 128 : (c + 1) * 128],
                vT[c][:, tb * 128 : (tb + 1) * 128],
                ident_h,
            )
        f_sb = work.tile([128, D], F16, name="f_sb", tag="f_sb", bufs=2)
        nc.scalar.copy(f_sb, ftp)
        o_sb = work.tile([128, D], F32, name="o_sb", tag="o_sb", bufs=2)
        nc.vector.tensor_tensor(o_sb, op_ps, f_sb, op=OP.add)
        nc.sync.dma_start(out=out_d[tb * 128 : (tb + 1) * 128, :], in_=o_sb)

    for qp in range(2):  # query-block pairs of 1024
        for h in range(H):
            att_pair(h, qp)
        if os.environ.get("SANM_EPI", "end") == "interleave":
            for tb8 in range(8):  # this pair's 128-token out blocks
                out_block(qp * 8 + tb8)
    if os.environ.get("SANM_EPI", "end") != "interleave":
        for tb in range(NT):
            out_block(tb)

    if os.environ.get("SANM_DEBUG", "0") == "1":
        dbg_q = nc.dram_tensor("dbg_q", (H, 128, T), BF16, kind="ExternalOutput").ap()
        dbg_v = nc.dram_tensor("dbg_v", (NC, 128, T), F16, kind="ExternalOutput").ap()
        dbg_c = nc.dram_tensor("dbg_c", (H, 128, T), BF16, kind="ExternalOutput").ap()
        dbg_k = nc.dram_tensor("dbg_k", (H, 128, TK), BF16, kind="ExternalOutput").ap()
        for hh in range(H):
            nc.sync.dma_start(out=dbg_q[hh], in_=qT[hh])
            nc.sync.dma_start(out=dbg_c[hh], in_=ctxT[hh])
            nc.sync.dma_start(out=dbg_k[hh], in_=kTc[hh])
            nc.sync.dma_start(out=dbg_v[hh], in_=vT[hh])

    main_cm.__exit__(None, None, None)
    stack.close()


_CACHE = {}
_FN_CACHE = {}


def make_sharded_fn(nc, n_cores=NCORES):
    """Build a reusable jitted executable for `nc` (done once per build).

    run_bass_kernel_spmd creates a fresh jax.jit per call, so every
    invocation re-traces, re-lowers and re-loads the NEFF; caching the
    jitted callable makes repeat kernel() calls cost only transfer+exec.
    """
    import jax
    from jax.experimental.shard_map import shard_map
    from jax.sharding import Mesh, PartitionSpec

    from concourse import bass2jax
    from concourse.bass2jax import _bass_exec_p, install_neuronx_cc_hook

    install_neuronx_cc_hook()
    partition_name = nc.partition_id_tensor.name if nc.partition_id_tensor else None
    in_names, out_names, out_avals, zero_outs = [], [], [], []
    for alloc in nc.m.functions[0].allocations:
        if not isinstance(alloc, mybir.MemoryLocationSet):
            continue
        name = alloc.memorylocations[0].name
        if alloc.kind == "ExternalInput":
            if name != partition_name:
                in_names.append(name)
        elif alloc.kind == "ExternalOutput":
            out_names.append(name)
            shape = tuple(alloc.tensor_shape)
            dtype = mybir.dt.np(alloc.dtype)
            out_avals.append(jax.core.ShapedArray(shape, dtype))
            zero_outs.append(np.zeros(shape, dtype))
    n_params = len(in_names)
    all_in_names = list(in_names) + list(out_names)
    if partition_name is not None:
        all_in_names.append(partition_name)

    def _body(*args):
        operands = list(args)
        if partition_name is not None:
            operands.append(bass2jax.partition_id_tensor())
        outs = _bass_exec_p.bind(
            *operands,
            out_avals=tuple(out_avals),
            in_names=tuple(all_in_names),
            out_names=tuple(out_names),
            lowering_input_output_aliases=(),
            sim_require_finite=True,
            sim_require_nnan=True,
            nc=nc,
        )
        return tuple(outs)

    devices = jax.devices()[:n_cores]
    mesh = Mesh(np.asarray(devices), ("core",))
    n_outs = len(out_avals)
    in_specs = (PartitionSpec("core"),) * (n_params + n_outs)
    out_specs = (PartitionSpec("core"),) * n_outs
    fn = jax.jit(
        shard_map(
            _body, mesh=mesh, in_specs=in_specs, out_specs=out_specs, check_rep=False
        ),
        keep_unused=True,
    )
    return fn, in_names, out_names, zero_outs


def run_cached(nc, in_maps, key):
    """Execute via a cached jitted executable (falls back to the slow path)."""
    import jax

    if key not in _FN_CACHE:
        _FN_CACHE[key] = make_sharded_fn(nc)
    fn, in_names, out_names, zero_outs = _FN_CACHE[key]
    n = len(in_maps)
    concat_in = [
        np.concatenate([np.asarray(in_maps[c][name]) for c in range(n)], axis=0)
        for name in in_names
    ]
    concat_zeros = [
        np.zeros((n * z.shape[0], *z.shape[1:]), z.dtype) for z in zero_outs
    ]
    out_arrs = fn(*concat_in, *concat_zeros)
    outs = [np.asarray(a) for a in out_arrs]
    return [
        {
            name: outs[i].reshape(n, outs[i].shape[0] // n, *outs[i].shape[1:])[c]
            for i, name in enumerate(out_names)
        }
        for c in range(n)
    ]


def _build(TK):
    key = (QKV_DT, ATT_DT, REPS, TK, NORM, FSMN_DT, LOOP)
    if key in _CACHE:
        return _CACHE[key]
    nc = bacc.Bacc(
        "TRN2",
        target_bir_lowering=False,
        debug=False,
        enable_asserts=False,
        num_devices=NCORES,
    )
    aps = (
        nc.dram_tensor("x", (T, D), BF16, kind="ExternalInput").ap(),
        nc.dram_tensor("mask", (T,), F32, kind="ExternalInput").ap(),
        nc.dram_tensor("xc", (TK, D), BF16, kind="ExternalInput").ap(),
        nc.dram_tensor("cbias", (TK,), F32, kind="ExternalInput").ap(),
        nc.dram_tensor("Wqkv", (D, 3 * D), BF16, kind="ExternalInput").ap(),
        nc.dram_tensor("bqkv", (3 * D,), F32, kind="ExternalInput").ap(),
        nc.dram_tensor("Wout", (D, D), BF16, kind="ExternalInput").ap(),
        nc.dram_tensor("bout", (D,), F32, kind="ExternalInput").ap(),
        nc.dram_tensor("fsmn_w", (D, 1, KS), F32, kind="ExternalInput").ap(),
        nc.dram_tensor("out", (T, D), F32, kind="ExternalOutput").ap(),
    )
    with tile.TileContext(nc) as tc:
        if LOOP > 0:
            # hw loop: NEFF size is constant in trip count, so a large trip
            # count isolates per-rep device time from dispatch overhead
            with tc.For_i(0, LOOP, 1):
                build_kernel_body(tc, aps, TK, 0)
        else:
            for rep in range(REPS):
                build_kernel_body(tc, aps, TK, rep)
    nc.compile()
    _CACHE[key] = nc
    return nc


def _bf16(a):
    import ml_dtypes

    return np.ascontiguousarray(a.astype(ml_dtypes.bfloat16))


def _compact(x_b, mask_b, TK):
    """Host-side gather of unmasked token rows, padded to TK (bf16 in/out)."""
    idx = np.nonzero(mask_b != 0)[0]
    n = len(idx)
    xc = np.zeros((TK, x_b.shape[1]), x_b.dtype)
    xc[:n] = x_b[idx[:TK]]
    cb = np.full((TK,), MASK_NEG, np.float32)
    cb[:n] = 0.0
    return xc, cb


def kernel(x, mask, Wqkv, bqkv, Wout, bout, fsmn_w):
    x = _bf16(np.asarray(x))
    mask = np.ascontiguousarray(np.asarray(mask, dtype=np.float32))
    Wqkv = _bf16(np.asarray(Wqkv))
    bqkv = np.ascontiguousarray(np.asarray(bqkv, dtype=np.float32))
    Wout = _bf16(np.asarray(Wout))
    bout = np.ascontiguousarray(np.asarray(bout, dtype=np.float32))
    fsmn_w = np.ascontiguousarray(np.asarray(fsmn_w, dtype=np.float32))

    counts = [int((mask[b, 0] != 0).sum()) for b in range(NCORES)]
    TK = min(T, max(256, int(-(-max(counts) // 128) * 128)))

    nc = _build(TK)
    in_maps = []
    for b in range(NCORES):
        xc, cb = _compact(x[b], mask[b, 0], TK)
        in_maps.append(
            {
                "x": x[b],
                "mask": np.ascontiguousarray(mask[b, 0]),
                "xc": xc,
                "cbias": cb,
                "Wqkv": Wqkv,
                "bqkv": bqkv,
                "Wout": Wout,
                "bout": bout,
                "fsmn_w": fsmn_w,
            }
        )
    try:
        results = run_cached(nc, in_maps, key=(id(nc), TK))
    except Exception:
        res = bass_utils.run_bass_kernel_spmd(
            nc, in_maps, core_ids=list(range(NCORES)), trace=False
        )
        results = res.results
    out = np.stack([results[b]["out"] for b in range(NCORES)], axis=0)
    return out


if __name__ == "__main__":
    rng = np.random.default_rng(0)
    ins = {
        "x": rng.standard_normal((NCORES, T, D), dtype=np.float32),
        "mask": rng.integers(0, 2, (NCORES, 1, T)).astype(np.float32),
        "Wqkv": (rng.standard_normal((D, 3 * D)) * 0.02).astype(np.float32),
        "bqkv": np.zeros((3 * D,), np.float32),
        "Wout": (rng.standard_normal((D, D)) * 0.02).astype(np.float32),
        "bout": np.zeros((D,), np.float32),
        "fsmn_w": (rng.standard_normal((D, 1, KS)) * 0.1).astype(np.float32),
    }
    out = kernel(**ins)
    print(out.shape, out.dtype, float(np.abs(out).max()))



# revision 81
# speedup vs baseline: 1.1897x; 1.0063x over previous
"""Trainium2 Bass kernel for MultiHeadedAttentionSANM.

Per-core (data-parallel over batch, 8 cores, B=1 each). Inputs x/xc/Wqkv/Wout
are pre-converted to bf16 on the host (abs tolerance budget allows it; rel
err ~2.4e-3 vs the 2e-2 gate).

  - x^T / xc^T via XBAR DMA transposes (2-byte mode, 14ns per 16x128 tile);
    xc is the host-side gather of the ~50% mask=1 tokens, padded to TK.
  - qkv^T = (x @ Wqkv)^T on PE (bf16): q^T and v^T on full tokens (FSMN
    needs all of v), k^T and attention-v^T on compact tokens only. v is
    projected FIRST so the FSMN conv can run on DVE during the remaining
    PE-heavy projections.
  - FSMN: depthwise conv over time in (d, t) layout, fp16 on DVE (fp16 keeps
    the accumulation rounding at the residual's scale 8x below bf16; the
    3-operand STT op has no fast DVE ucode mode, so each tap is a 4x-mode
    tensor_scalar mult plus a 2x-mode add). Ops are yielded one at a time
    and pulled between projection blocks so the in-order DVE queue never
    blocks attention work behind the conv.
  - attention: scores transposed (compact keys on partitions) so the exp
    output feeds the ctx matmul directly as the rhs; padded keys get a
    -30000 exp bias. Two 512-query blocks share every PE stationary load.
    Z = sum_k exp via a DVE bf16 chain plus PE ones-matmuls (the same bf16
    exp values are summed for numerator and denominator, so their rounding
    cancels); 1/Z is broadcast across partitions with a PE outer product.
  - epilogue: out-proj with ctxT stationary per (head, t-block); the FSMN
    result is PE-transposed back to natural layout and fused into the add.

Timing protocol: the whole body can run inside a hardware For_i loop
(SANM_LOOP) so a large on-device trip count isolates per-rep time from the
~100ms axon dispatch round trip; see test.py.
"""

import os
import sys

for _p in ("/opt/trn_rl_repo", "/root/.axon_site/_ro/trn_rl_repo"):
    if os.path.isdir(_p) and _p not in sys.path:
        sys.path.append(_p)

from contextlib import ExitStack

import numpy as np

import concourse.bass as bass
import concourse.mybir as mybir
import concourse.tile as tile
from concourse import bacc
from concourse import bass_utils
from concourse.masks import make_identity

T, D, H, DK, KS, PAD = 2048, 512, 4, 128, 11, 5
NCORES = 8
NT = T // 128          # 16 t-blocks of 128
NC = D // 128          # 4 d-chunks of 128
SCALE = float(DK) ** -0.5
MASK_NEG = -30000.0

F32 = mybir.dt.float32
F32R = mybir.dt.float32r
BF16 = mybir.dt.bfloat16
F16 = mybir.dt.float16
AF = mybir.ActivationFunctionType
OP = mybir.AluOpType

REPS = int(os.environ.get("SANM_REPS", "1"))     # timing: repeat body in one NEFF
LOOP = int(os.environ.get("SANM_LOOP", "0"))     # timing: hw For_i loop trip count


def _bcast_vec(ap, nrows):
    """Broadcast a flat [N] DRAM AP across partitions -> [nrows, N]."""
    return bass.AP(tensor=ap.tensor, offset=ap.offset, ap=[[0, nrows]] + list(ap.ap))


def _tiles(total, step=512):
    out, p = [], 0
    while p < total:
        n = min(step, total - p)
        rem = total - p - n
        if 0 < rem < 256:  # avoid <256-wide f32r tails (4 cyc/row penalty)
            n = (n + rem) // 2
            n = (n + 127) // 128 * 128
        out.append((p, n))
        p += n
    return out


def build_kernel_body(tc, aps, TK, rep=0):
    nc = tc.nc
    x_d, mask_d, xc_d, cbias_d, wqkv_d, bqkv_d, wout_d, bout_d, fw_d, out_d = aps
    R = f"r{rep}_" if rep else ""
    TKC = TK // 128  # compact key chunks

    stack = ExitStack()
    consts = stack.enter_context(tc.tile_pool(name=R + "consts", bufs=1))
    work = stack.enter_context(tc.tile_pool(name=R + "work", bufs=2))
    ps = stack.enter_context(tc.tile_pool(name=R + "ps", bufs=1, space="PSUM"))

    # p_main holds all long-lived tensors (whole kernel); p_x nests inside it
    # (LIFO) and is released after the qkv matmuls to reclaim x^T space.
    main_cm = tc.tile_pool(name=R + "p_main", bufs=1)
    x_cm = tc.tile_pool(name=R + "p_x", bufs=1)
    p_main = main_cm.__enter__()
    p_x = x_cm.__enter__()

    # ---------------- constants ----------------
    ident = consts.tile([128, 128], F32, name="ident", tag="ident")
    make_identity(nc, ident)
    ident_b = consts.tile([128, 128], BF16, name="ident_b", tag="ident_b")
    nc.vector.tensor_copy(ident_b, ident)
    ident_h = consts.tile([128, 128], F16, name="ident_h", tag="ident_h")
    nc.vector.tensor_copy(ident_h, ident)

    ones_att = consts.tile([128, 1], BF16, name="ones_att", tag="ones_att")
    nc.vector.memset(ones_att, 1.0)
    ones_row = consts.tile([1, 128], BF16, name="ones_row", tag="ones_row")
    nc.vector.memset(ones_row, 1.0)

    # ---------------- x^T and xc^T (XBAR DMA transposes) ---------------------
    # x/xc arrive bf16 (host pre-converts); the DMA engines' 2-byte transpose
    # mode (14ns per 16x128 tile) replaces the load+PE-transpose+copy pipeline.
    # x first (it gates the v projection -> FSMN chain); consts ride SWDGE.
    xT = p_x.tile([128, NC, T], BF16, name="xT", tag="xT")
    xcT = p_x.tile([128, NC, TK], BF16, name="xcT", tag="xcT")
    for c in range(NC):
        (nc.sync if c % 2 == 0 else nc.scalar).dma_start(
            out=xT[:, c, :], in_=x_d[:, c * 128 : (c + 1) * 128], transpose=True
        )

    def xc_transposes():
        for c in range(NC):
            (nc.sync if c % 2 == 0 else nc.scalar).dma_start(
                out=xcT[:, c, :], in_=xc_d[:, c * 128 : (c + 1) * 128], transpose=True
            )

    # biases as per-partition columns (bq gates the first sinks — load first)
    bq = consts.tile([128, 12], F32, name="bq", tag="bq")
    nc.gpsimd.dma_start(out=bq, in_=bqkv_d.rearrange("(c p) -> p c", p=128))

    # remaining consts are emitted just before first use (see load_late_consts)
    mbias = consts.tile([128, TKC], F32, name="mbias", tag="mbias")
    mrow = consts.tile([128, T], BF16, name="mrow", tag="mrow")
    bo = consts.tile([128, NC], F32, name="bo", tag="bo")
    wadj = consts.tile([128, NC, KS], F32, name="wadj", tag="wadj")
    wo = consts.tile([128, NC, D], BF16, name="wo", tag="wo")

    def load_fsmn_consts():
        # mask broadcast (128, T) bf16 (exact for 0/1) + conv weights; needed
        # by the first FSMN pull, after the first v projection
        nc.gpsimd.dma_start(out=mrow, in_=_bcast_vec(mask_d, 128))
        nc.gpsimd.dma_start(
            out=wadj, in_=fw_d.rearrange("(c p) o k -> p c (o k)", p=128)
        )
        nc.vector.tensor_scalar_add(
            wadj[:, :, PAD : PAD + 1], wadj[:, :, PAD : PAD + 1], 1.0
        )
        nc.gpsimd.dma_start(out=bo, in_=bout_d.rearrange("(c p) -> p c", p=128))

    def load_late_consts():
        # compact-key exp bias (0 valid / -30000 padded), columns (128, TKC)
        nc.gpsimd.dma_start(out=mbias, in_=cbias_d.rearrange("(c p) -> p c", p=128))
        # Wout (128, NC, 512) bf16, loaded directly (host pre-converts)
        nc.gpsimd.dma_start(out=wo, in_=wout_d.rearrange("(c p) d -> p c d", p=128))

    # ---------------- qkv^T = (x @ Wqkv)^T ----------------
    # q on full tokens; k only compact; v full (FSMN) and compact (attention)
    qT = [p_main.tile([128, T], BF16, name=f"qT{h}", tag=f"qT{h}") for h in range(H)]
    kTc = [p_main.tile([128, TK], BF16, name=f"kTc{h}", tag=f"kTc{h}") for h in range(H)]
    # fp16 for the FSMN path: same 2-byte DVE speed, 8x finer mantissa (the
    # conv accumulates at the residual's scale, where bf16 rounding is ~1e-2)
    vT = [p_main.tile([128, T], F16, name=f"vT{c}", tag=f"vT{c}") for c in range(NC)]
    vcT = [p_main.tile([128, TK], BF16, name=f"vcT{c}", tag=f"vcT{c}") for c in range(NC)]

    def project(f, srcT, tspans, sink):
        """psum[128, n] = Wqkv[:, f-block].T @ src over d-chunks, then sink.

        span-outer so each span's sink (ACT) pipelines behind the next
        span's matmuls instead of bunching at the end of the f-block."""
        wqf = work.tile([128, NC, 128], BF16, name="wqf", tag="wqf", bufs=3)
        wqf_src = wqkv_d[:, f * 128 : (f + 1) * 128].rearrange(
            "(c p) f -> p c f", p=128
        )
        nc.gpsimd.dma_start(out=wqf, in_=wqf_src)
        for t0, n in tspans:
            mm = ps.tile([128, 512], F32, name="mmq", tag="s", bufs=4)
            for dc in range(NC):
                nc.tensor.matmul(
                    mm[:, :n],
                    wqf[:, dc, :],
                    srcT[:, dc, t0 : t0 + n],
                    start=(dc == 0),
                    stop=(dc == NC - 1),
                )
            sink(mm, t0, n)

    def act_sink(dst, f):
        def sink(mm, t0, n):
            nc.scalar.activation(
                dst[:, t0 : t0 + n], mm[:, :n], AF.Identity,
                bias=bq[:, f : f + 1], scale=1.0,
            )
        return sink

    # -------- FSMN op stream (d, t layout), fp16, all on DVE -----------------
    # v is projected FIRST so the FSMN conv can run on DVE throughout the
    # PE-heavy q/k/vc projection phase; ops are yielded one at a time and
    # pulled between projection blocks / attention blocks
    vmt = [p_main.tile([128, T], F16, name=f"vmt{c}", tag=f"vmt{c}") for c in range(NC)]
    fac = [p_main.tile([128, T], F16, name=f"fac{c}", tag=f"fac{c}") for c in range(NC)]

    def _fsmn_ops():
        # STT (mult+add) has no fast DVE ucode mode, so each tap is a
        # 4x-mode tensor_scalar mult into a scratch plus a 2x-mode add.
        # Yields one DVE op at a time so the attention loop can interleave
        # them finely and the in-order DVE queue never falls behind.
        for c in range(NC):
            vm, acc = vmt[c], fac[c]
            yield lambda c=c, vm=vm: nc.vector.tensor_tensor(
                vm, vT[c], mrow, op=OP.mult
            )
            yield lambda c=c, vm=vm, acc=acc: nc.vector.tensor_scalar_mul(
                acc, vm, wadj[:, c, PAD : PAD + 1]
            )
            for kk in list(range(0, PAD)) + list(range(PAD + 1, KS)):
                s = kk - PAD
                lo, hi = max(0, -s), T - max(0, s)

                def tapop(c=c, vm=vm, acc=acc, kk=kk, lo=lo, hi=hi, s=s):
                    tap = work.tile([128, T], F16, name="tap", tag="tap", bufs=2)
                    nc.vector.tensor_scalar_mul(
                        tap[:, lo:hi], vm[:, lo + s : hi + s], wadj[:, c, kk : kk + 1]
                    )
                    nc.vector.tensor_tensor(
                        acc[:, lo:hi], acc[:, lo:hi], tap[:, lo:hi], op=OP.add
                    )

                yield tapop
            # out = (conv + vm) * m + bo (center tap carries the +1 residual)
            yield lambda c=c, acc=acc: nc.vector.tensor_tensor(
                vT[c], acc, mrow, op=OP.mult
            )
            yield lambda c=c: nc.vector.tensor_scalar_add(
                vT[c], vT[c], bo[:, c : c + 1]
            )

    fsmn_iter = _fsmn_ops()

    def pull_fsmn(k):
        for _ in range(k):
            op = next(fsmn_iter, None)
            if op is None:
                return
            op()

    for i, f in enumerate(range(8, 12)):  # v full FIRST (unblocks FSMN on DVE)
        project(f, xT, _tiles(T), act_sink(vT[f - 8], f))
        if i == 0:
            load_fsmn_consts()
            xc_transposes()  # off the startup-critical path
        else:
            pull_fsmn(14)  # chunk i-1 (vT[i-1] is fully sinked by now)
    load_late_consts()
    for f in range(4):  # q: full tokens -> qT (bf16), bias via ACT
        project(f, xT, _tiles(T), act_sink(qT[f], f))
        pull_fsmn(4)
    for f in range(4, 8):  # k: compact tokens -> kTc
        project(f, xcT, _tiles(TK), act_sink(kTc[f - 4], f))
    for f in range(8, 12):  # v compact tokens (attention)
        project(f, xcT, _tiles(TK), act_sink(vcT[f - 8], f))
    x_cm.__exit__(None, None, None)  # frees xT, xcT
    pull_fsmn(99)  # any remainder: epilogue blocks interleave with attention

    # ------------- compact v natural (PE transposes of vcT, batched) ---------
    vh = [
        p_main.tile([128, TKC, 128], BF16, name=f"vh{h}", tag=f"vh{h}")
        for h in range(H)
    ]
    for h in range(H):
        for j0 in range(0, TKC, 4):
            jn = min(4, TKC - j0)
            tp = ps.tile([128, 512], BF16, name="tpv", tag="s", bufs=4)
            for j in range(jn):
                nc.tensor.transpose(
                    tp[:, j * 128 : (j + 1) * 128],
                    vcT[h][:, (j0 + j) * 128 : (j0 + j + 1) * 128],
                    ident_b,
                )
            nc.scalar.copy(vh[h][:, j0 : j0 + jn, :], tp[:, : jn * 128])

    # ---------------- attention + interleaved epilogue ----------------
    # per (query-block of 512, head): scores transposed (compact keys on
    # partitions); exp with -30000 pad bias; ctx accumulates over key chunks
    # in one PSUM bank; Z = sum_k exp via a DVE/Pool add tree + one
    # ones-matmul; 1/Z broadcast across partitions via a DRAM bounce;
    # normalize on DVE. qb is the OUTER loop so each query block's out
    # projection runs as soon as its 4 heads are done (no serial tail).
    ctxT = [
        p_main.tile([128, T], BF16, name=f"ctxT{h}", tag=f"ctxT{h}")
        for h in range(H)
    ]

    def att_pair(h, qp):
        """Two 512-query blocks share every PE stationary load (kTc, vh,
        ones): halves the attention ldweights count."""
        ia, ib = qp * 1024, qp * 1024 + 512
        ctx_a = ps.tile([128, 512], F32, name="ctx_a", tag="actx", bufs=3)
        ctx_b = ps.tile([128, 512], F32, name="ctx_b", tag="actx", bufs=3)
        esum_a = work.tile([128, 512], BF16, name="esum_a", tag="esum", bufs=4)
        esum_b = work.tile([128, 512], BF16, name="esum_b", tag="esum", bufs=4)
        last_e = []
        for jc in range(TKC):
            kT_j = kTc[h][:, jc * 128 : (jc + 1) * 128]
            s_a = ps.tile([128, 512], F32, name="s_a", tag="s", bufs=4)
            s_b = ps.tile([128, 512], F32, name="s_b", tag="s", bufs=4)
            nc.tensor.matmul(
                s_a, kT_j, qT[h][:, ia : ia + 512],
                start=True, stop=True, skip_group_check=True,
            )
            nc.tensor.matmul(
                s_b, kT_j, qT[h][:, ib : ib + 512],
                start=True, stop=True, skip_group_check=True,
            )
            e_a = work.tile([128, 512], BF16, name="e_a", tag="eT", bufs=4)
            e_b = work.tile([128, 512], BF16, name="e_b", tag="eT", bufs=4)
            nc.scalar.activation(
                e_a, s_a, AF.Exp, bias=mbias[:, jc : jc + 1], scale=SCALE
            )
            nc.scalar.activation(
                e_b, s_b, AF.Exp, bias=mbias[:, jc : jc + 1], scale=SCALE
            )
            vh_j = vh[h][:, jc, :]
            nc.tensor.matmul(
                ctx_a, vh_j, e_a,
                start=(jc == 0), stop=(jc == TKC - 1), skip_group_check=True,
            )
            nc.tensor.matmul(
                ctx_b, vh_j, e_b,
                start=(jc == 0), stop=(jc == TKC - 1), skip_group_check=True,
            )
            # Z = sum_k exp: DVE accumulates the first 7 chunks (bf16 2x mode;
            # attention is ~1% of |out| so bf16 partials are plenty), the last
            # two ride PE ones-matmuls below to balance engine load
            if jc == 0:
                nc.vector.tensor_copy(esum_a, e_a)
                nc.vector.tensor_copy(esum_b, e_b)
            elif jc < TKC - 2:
                nc.vector.tensor_tensor(esum_a, esum_a, e_a, op=OP.add)
                nc.vector.tensor_tensor(esum_b, esum_b, e_b, op=OP.add)
            else:
                last_e.append((e_a, e_b))
        for half, (i0, esum_d) in enumerate(((ia, esum_a), (ib, esum_b))):
            z_ps = ps.tile([1, 512], F32, name="z_ps", tag="z", bufs=1)
            for i, e_pair in enumerate(last_e):
                nc.tensor.matmul(
                    z_ps, ones_att, e_pair[half],
                    start=(i == 0), stop=False, skip_group_check=True,
                )
            nc.tensor.matmul(
                z_ps, ones_att, esum_d, start=False, stop=True,
                skip_group_check=True,
            )
            rz = work.tile([1, 512], BF16, name="rz", tag="rz", bufs=2)
            with nc.allow_low_precision(reason="1/Z applied to bf16 weights"):
                nc.vector.reciprocal(rz, z_ps)
            # broadcast 1/Z across partitions as a PE outer product; shares
            # the z bank (the z->recip->broadcast chain is strictly serial)
            zb_ps = ps.tile([128, 512], F32, name="zb_ps", tag="z", bufs=1)
            nc.tensor.matmul(
                zb_ps, ones_row, rz, start=True, stop=True, skip_group_check=True
            )
            zb_sb = work.tile([128, 512], BF16, name="zb_sb", tag="zb_sb", bufs=2)
            nc.scalar.copy(zb_sb, zb_ps)
            ctx = ctx_a if half == 0 else ctx_b
            nc.vector.tensor_tensor(
                ctxT[h][:, i0 : i0 + 512], ctx, zb_sb, op=OP.mult
            )

    def out_block(tb):
        op_ps = ps.tile([128, 512], F32, name="op_ps", tag="actx", bufs=3)
        for h in range(H):
            nc.tensor.matmul(
                op_ps,
                ctxT[h][:, tb * 128 : (tb + 1) * 128],
                wo[:, h, :],
                start=(h == 0),
                stop=(h == H - 1),
            )
        # transpose this t-block of fsmn into natural layout (fp16, 1 cyc/row)
        ftp = ps.tile([128, 512], F16, name="ftp", tag="z", bufs=1)
        for c in range(NC):
            nc.tensor.transpose(
                ftp[:, c * 128 : (c + 1) * 128],
                vT[c][:, tb * 128 : (tb + 1) * 128],
                ident_h,
            )
        f_sb = work.tile([128, D], F16, name="f_sb", tag="f_sb", bufs=2)
        nc.scalar.copy(f_sb, ftp)
        o_sb = work.tile([128, D], F32, name="o_sb", tag="o_sb", bufs=2)
        nc.vector.tensor_tensor(o_sb, op_ps, f_sb, op=OP.add)
        nc.sync.dma_start(out=out_d[tb * 128 : (tb + 1) * 128, :], in_=o_sb)

    for qp in range(2):  # query-block pairs of 1024
        for h in range(H):
            att_pair(h, qp)
        if os.environ.get("SANM_EPI", "end") == "interleave":
            for tb8 in range(8):  # this pair's 128-token out blocks
                out_block(qp * 8 + tb8)
    if os.environ.get("SANM_EPI", "end") != "interleave":
        for tb in range(NT):
            out_block(tb)

    if os.environ.get("SANM_DEBUG", "0") == "1":
        dbg_q = nc.dram_tensor("dbg_q", (H, 128, T), BF16, kind="ExternalOutput").ap()
        dbg_v = nc.dram_tensor("dbg_v", (NC, 128, T), F16, kind="ExternalOutput").ap()
        dbg_c = nc.dram_tensor("dbg_c", (H, 128, T), BF16, kind="ExternalOutput").ap()
        dbg_k = nc.dram_tensor("dbg_k", (H, 128, TK), BF16, kind="ExternalOutput").ap()
        for hh in range(H):
            nc.sync.dma_start(out=dbg_q[hh], in_=qT[hh])
            nc.sync.dma_start(out=dbg_c[hh], in_=ctxT[hh])
            nc.sync.dma_start(out=dbg_k[hh], in_=kTc[hh])
            nc.sync.dma_start(out=dbg_v[hh], in_=vT[hh])

    main_cm.__exit__(None, None, None)
    stack.close()


_CACHE = {}
_FN_CACHE = {}


def make_sharded_fn(nc, n_cores=NCORES):
    """Build a reusable jitted executable for `nc` (done once per build).

    run_bass_kernel_spmd creates a fresh jax.jit per call, so every
    invocation re-traces, re-lowers and re-loads the NEFF; caching the
    jitted callable makes repeat kernel() calls cost only transfer+exec.
    """
    import jax
    from jax.experimental.shard_map import shard_map
    from jax.sharding import Mesh, PartitionSpec

    from concourse import bass2jax
    from concourse.bass2jax import _bass_exec_p, install_neuronx_cc_hook

    install_neuronx_cc_hook()
    partition_name = nc.partition_id_tensor.name if nc.partition_id_tensor else None
    in_names, out_names, out_avals, zero_outs = [], [], [], []
    for alloc in nc.m.functions[0].allocations:
        if not isinstance(alloc, mybir.MemoryLocationSet):
            continue
        name = alloc.memorylocations[0].name
        if alloc.kind == "ExternalInput":
            if name != partition_name:
                in_names.append(name)
        elif alloc.kind == "ExternalOutput":
            out_names.append(name)
            shape = tuple(alloc.tensor_shape)
            dtype = mybir.dt.np(alloc.dtype)
            out_avals.append(jax.core.ShapedArray(shape, dtype))
            zero_outs.append(np.zeros(shape, dtype))
    n_params = len(in_names)
    all_in_names = list(in_names) + list(out_names)
    if partition_name is not None:
        all_in_names.append(partition_name)

    def _body(*args):
        operands = list(args)
        if partition_name is not None:
            operands.append(bass2jax.partition_id_tensor())
        outs = _bass_exec_p.bind(
            *operands,
            out_avals=tuple(out_avals),
            in_names=tuple(all_in_names),
            out_names=tuple(out_names),
            lowering_input_output_aliases=(),
            sim_require_finite=True,
            sim_require_nnan=True,
            nc=nc,
        )
        return tuple(outs)

    devices = jax.devices()[:n_cores]
    mesh = Mesh(np.asarray(devices), ("core",))
    n_outs = len(out_avals)
    in_specs = (PartitionSpec("core"),) * (n_params + n_outs)
    out_specs = (PartitionSpec("core"),) * n_outs
    fn = jax.jit(
        shard_map(
            _body, mesh=mesh, in_specs=in_specs, out_specs=out_specs, check_rep=False
        ),
        keep_unused=True,
    )
    return fn, in_names, out_names, zero_outs


def run_cached(nc, in_maps, key):
    """Execute via a cached jitted executable (falls back to the slow path)."""
    import jax

    if key not in _FN_CACHE:
        _FN_CACHE[key] = make_sharded_fn(nc)
    fn, in_names, out_names, zero_outs = _FN_CACHE[key]
    n = len(in_maps)
    concat_in = [
        np.concatenate([np.asarray(in_maps[c][name]) for c in range(n)], axis=0)
        for name in in_names
    ]
    concat_zeros = [
        np.zeros((n * z.shape[0], *z.shape[1:]), z.dtype) for z in zero_outs
    ]
    out_arrs = fn(*concat_in, *concat_zeros)
    outs = [np.asarray(a) for a in out_arrs]
    return [
        {
            name: outs[i].reshape(n, outs[i].shape[0] // n, *outs[i].shape[1:])[c]
            for i, name in enumerate(out_names)
        }
        for c in range(n)
    ]


def _build(TK):
    key = (REPS, TK, LOOP)
    if key in _CACHE:
        return _CACHE[key]
    nc = bacc.Bacc(
        "TRN2",
        target_bir_lowering=False,
        debug=False,
        enable_asserts=False,
        num_devices=NCORES,
    )
    aps = (
        nc.dram_tensor("x", (T, D), BF16, kind="ExternalInput").ap(),
        nc.dram_tensor("mask", (T,), F32, kind="ExternalInput").ap(),
        nc.dram_tensor("xc", (TK, D), BF16, kind="ExternalInput").ap(),
        nc.dram_tensor("cbias", (TK,), F32, kind="ExternalInput").ap(),
        nc.dram_tensor("Wqkv", (D, 3 * D), BF16, kind="ExternalInput").ap(),
        nc.dram_tensor("bqkv", (3 * D,), F32, kind="ExternalInput").ap(),
        nc.dram_tensor("Wout", (D, D), BF16, kind="ExternalInput").ap(),
        nc.dram_tensor("bout", (D,), F32, kind="ExternalInput").ap(),
        nc.dram_tensor("fsmn_w", (D, 1, KS), F32, kind="ExternalInput").ap(),
        nc.dram_tensor("out", (T, D), F32, kind="ExternalOutput").ap(),
    )
    with tile.TileContext(nc) as tc:
        if LOOP > 0:
            # hw loop: NEFF size is constant in trip count, so a large trip
            # count isolates per-rep device time from dispatch overhead
            with tc.For_i(0, LOOP, 1):
                build_kernel_body(tc, aps, TK, 0)
        else:
            for rep in range(REPS):
                build_kernel_body(tc, aps, TK, rep)
    nc.compile()
    _CACHE[key] = nc
    return nc


def _bf16(a):
    import ml_dtypes

    return np.ascontiguousarray(a.astype(ml_dtypes.bfloat16))


def _compact(x_b, mask_b, TK):
    """Host-side gather of unmasked token rows, padded to TK (bf16 in/out)."""
    idx = np.nonzero(mask_b != 0)[0]
    n = len(idx)
    xc = np.zeros((TK, x_b.shape[1]), x_b.dtype)
    xc[:n] = x_b[idx[:TK]]
    cb = np.full((TK,), MASK_NEG, np.float32)
    cb[:n] = 0.0
    return xc, cb


def kernel(x, mask, Wqkv, bqkv, Wout, bout, fsmn_w):
    x = _bf16(np.asarray(x))
    mask = np.ascontiguousarray(np.asarray(mask, dtype=np.float32))
    Wqkv = _bf16(np.asarray(Wqkv))
    bqkv = np.ascontiguousarray(np.asarray(bqkv, dtype=np.float32))
    Wout = _bf16(np.asarray(Wout))
    bout = np.ascontiguousarray(np.asarray(bout, dtype=np.float32))
    fsmn_w = np.ascontiguousarray(np.asarray(fsmn_w, dtype=np.float32))

    counts = [int((mask[b, 0] != 0).sum()) for b in range(NCORES)]
    TK = min(T, max(256, int(-(-max(counts) // 128) * 128)))

    nc = _build(TK)
    in_maps = []
    for b in range(NCORES):
        xc, cb = _compact(x[b], mask[b, 0], TK)
        in_maps.append(
            {
                "x": x[b],
                "mask": np.ascontiguousarray(mask[b, 0]),
                "xc": xc,
                "cbias": cb,
                "Wqkv": Wqkv,
                "bqkv": bqkv,
                "Wout": Wout,
                "bout": bout,
                "fsmn_w": fsmn_w,
            }
        )
    try:
        results = run_cached(nc, in_maps, key=(id(nc), TK))
    except Exception:
        res = bass_utils.run_bass_kernel_spmd(
            nc, in_maps, core_ids=list(range(NCORES)), trace=False
        )
        results = res.results
    out = np.stack([results[b]["out"] for b in range(NCORES)], axis=0)
    return out


if __name__ == "__main__":
    rng = np.random.default_rng(0)
    ins = {
        "x": rng.standard_normal((NCORES, T, D), dtype=np.float32),
        "mask": rng.integers(0, 2, (NCORES, 1, T)).astype(np.float32),
        "Wqkv": (rng.standard_normal((D, 3 * D)) * 0.02).astype(np.float32),
        "bqkv": np.zeros((3 * D,), np.float32),
        "Wout": (rng.standard_normal((D, D)) * 0.02).astype(np.float32),
        "bout": np.zeros((D,), np.float32),
        "fsmn_w": (rng.standard_normal((D, 1, KS)) * 0.1).astype(np.float32),
    }
    out = kernel(**ins)
    print(out.shape, out.dtype, float(np.abs(out).max()))

